# revision 1
# baseline (speedup 1.0000x reference)
"""Longformer block-diagonal self-attention on 8 Trainium2 NeuronCores, v2.

Sharding: core = batch*2 + head_group.  Each core handles one batch (S=4096)
and 8 of the 16 heads (a 512-wide slice of the embedding).  Per-core output is
the partial out-projection (O_g @ Wo_g); host sums the two head-group partials
per batch and adds the constant row (bv @ Wo + bo).

v2 redesign vs v1: the attention inner loop computes S^T = K_w^T Q_w directly
(swapped matmul operands, same PE cost), so exp(S^T) is already key-major and
feeds the PV matmul as the stationary operand with no transpose.  The softmax
denominator comes from a ones-column appended to V (row sums ride along in the
PV accumulation), and the 1/sum normalization is applied after PV where the
query index is the partition dim (cheap tensor_scalar).  Token-major O is
turned back into feature-major via PE transposes (2 heads per 128x128 tile).
This removes every DMA-xbar transpose (~2.2us latency each) from the critical
path.

Algebraic simplifications (unchanged from v1):
  - bk drops out of softmax entirely; bv contributes bv @ Wo everywhere -> host.
  - 1/sqrt(head_dim) folded into Wq/bq on host.
"""

import numpy as np
import ml_dtypes

import concourse.bass as bass
import concourse.tile as tile
from concourse import bacc, mybir
from concourse.bass_utils import run_bass_kernel_spmd

F32 = mybir.dt.float32
BF16 = mybir.dt.bfloat16

B, S, E = 4, 4096, 1024
H, D, W = 8, 64, 256          # per-core heads, head dim, window
EG = H * D                    # 512: per-core embedding slice
CHUNK = 512
NCHUNK = S // CHUNK
NW_CHUNK = CHUNK // W         # windows per chunk (2)
KT_E = E // 128               # contraction tiles over E (8)
N_ETILE = EG // 128           # e'-tiles per group (4); head pair per tile

_NC_CACHE = {}

# tuning knobs (A/B-tested against the timeline sim)
CFG = {
    "fsb_dve": True,    # f_sb out-proj copies on DVE (else Act)
    "vsb_dve": False,   # v_sb copies on DVE (else Act)
    "ps_s_bufs": 2,
    "ps_t_bufs": 2,
    "ps_big_bufs": 2,
    "ph3_early": True,  # emit prev-chunk out-proj right after QK (else after scores+V)
    "sv_pattern": 0,    # 0: [sc(i,0) sc(i,1) V(i)]x4  1: [sc sc sc sc V V ...]  2: [sc(i,0) V(i) sc(i,1)]
    "store_sp": False,  # out-store DMAs on SP queue (else gpsimd/Pool)
    "exp_merge": True,  # one exp per (et,sub) over both kh (else two)
    "norm_split": False,  # head A on DVE + head B on Act (else both DVE)
    "attn_bufs": 3,
    "xpool_bufs": 2,
}


def _build_nc(niter=0):
    nc = bacc.Bacc("TRN2", target_bir_lowering=False, debug=False, num_devices=8)
    xt = nc.dram_tensor("xt", [E, S], BF16, kind="ExternalInput").ap()
    wq = nc.dram_tensor("wq", [E, EG], BF16, kind="ExternalInput").ap()
    wk = nc.dram_tensor("wk", [E, EG], BF16, kind="ExternalInput").ap()
    wv = nc.dram_tensor("wv", [E, EG], BF16, kind="ExternalInput").ap()
    wo = nc.dram_tensor("wo", [EG, E], BF16, kind="ExternalInput").ap()
    bq = nc.dram_tensor("bq", [EG, 1], F32, kind="ExternalInput").ap()
    out = nc.dram_tensor("out", [S, E], F32, kind="ExternalOutput").ap()

    with tile.TileContext(nc) as tc:
        _body(tc, nc, xt, wq, wk, wv, wo, bq, out, niter)
    nc.compile()
    return nc


def _emit_ph3(nc, pools, wts, out, prev):
    consts, xpool, qkv, attn, otp, fo, ps_big, ps_s, ps_o, ps_t = pools
    wq_t, wk_t, wv_t, wo_t, bq_t, identb = wts
    ot_big, s0 = prev
    for t in range(CHUNK // 128):
        f_sb = fo.tile([128, E], F32, name=f"f{t}", tag="fout")
        for eh in range(2):
            pf = ps_big.tile([128, 512], F32, name=f"pf{t}_{eh}", tag="big")
            for k4 in range(N_ETILE):
                nc.tensor.matmul(pf[:],
                                 ot_big[:, k4, t * 128:(t + 1) * 128],
                                 wo_t[k4][:, eh * 512:(eh + 1) * 512],
                                 start=(k4 == 0), stop=(k4 == N_ETILE - 1))
            if CFG["fsb_dve"]:
                nc.vector.tensor_copy(f_sb[:, eh * 512:(eh + 1) * 512], pf[:])
            else:
                nc.scalar.copy(f_sb[:, eh * 512:(eh + 1) * 512], pf[:])
        eng = nc.sync if CFG["store_sp"] else nc.gpsimd
        eng.dma_start(out[s0 + t * 128:s0 + (t + 1) * 128, :], f_sb[:])


def _chunk_body(tc, nc, pools, wts, xt, out, c, prev):
    consts, xpool, qkv, attn, otp, fo, ps_big, ps_s, ps_o, ps_t = pools
    wq_t, wk_t, wv_t, wo_t, bq_t, identb = wts
    s0 = c * CHUNK

    xc = []
    for k in range(KT_E):
        t = xpool.tile([128, CHUNK], BF16, name=f"xc{k}", tag=f"xc{k}")
        nc.gpsimd.dma_start(t[:], xt[k * 128:(k + 1) * 128, s0:s0 + CHUNK])
        xc.append(t)

    # ---- Q^T, K^T (e'-major) ----
    qt, kt = [], []
    for t in range(N_ETILE):
        pq = ps_big.tile([128, CHUNK], F32, name=f"pq{t}", tag="big")
        for k in range(KT_E):
            nc.tensor.matmul(pq[:], wq_t[k][:, t * 128:(t + 1) * 128],
                             xc[k][:], start=(k == 0), stop=(k == KT_E - 1))
        q_sb = qkv.tile([128, CHUNK], BF16, name=f"qt{t}", tag=f"qt{t}")
        nc.vector.tensor_scalar_add(q_sb[:], pq[:], bq_t[t][:])
        qt.append(q_sb)

        pk = ps_big.tile([128, CHUNK], F32, name=f"pk{t}", tag="big")
        for k in range(KT_E):
            nc.tensor.matmul(pk[:], wk_t[k][:, t * 128:(t + 1) * 128],
                             xc[k][:], start=(k == 0), stop=(k == KT_E - 1))
        k_sb = qkv.tile([128, CHUNK], BF16, name=f"kt{t}", tag=f"kt{t}")
        nc.scalar.copy(k_sb[:], pk[:])
        kt.append(k_sb)

    # ---- scores^T + exp, interleaved with V projection so the Act-engine
    # exp burst hides under PE matmuls ----
    p = {}
    vt = []

    def emit_scores(et, sub):
        prow = sub * 64
        for wl in range(NW_CHUNK):
            k0 = wl * W
            ps = ps_s.tile([128, 2, W], F32, name=f"s{et}{sub}{wl}", tag="sc")
            for kh in range(2):
                nc.tensor.matmul(
                    ps[:, kh, :],
                    kt[et][prow:prow + 64, k0 + kh * 128:k0 + (kh + 1) * 128],
                    qt[et][prow:prow + 64, k0:k0 + W],
                    start=True, stop=True)
            pt = attn.tile([128, 2, W], BF16, name=f"p{et}{sub}",
                           tag=f"p{et}{sub}")
            if CFG["exp_merge"]:
                nc.scalar.activation(pt[:], ps[:],
                                     mybir.ActivationFunctionType.Exp)
            else:
                for kh in range(2):
                    nc.scalar.activation(pt[:, kh, :], ps[:, kh, :],
                                         mybir.ActivationFunctionType.Exp)
            p[(wl, et, sub)] = pt

    def emit_v(t):
        pv = ps_big.tile([128, EG], F32, name=f"pv{t}", tag="big")
        for k in range(KT_E):
            nc.tensor.matmul(pv[:], xc[k][:, t * 128:(t + 1) * 128],
                             wv_t[k][:], start=(k == 0), stop=(k == KT_E - 1))
        v_sb = qkv.tile([128, H, D + 1], BF16, name=f"vt{t}", tag=f"vt{t}")
        nc.vector.memset(v_sb[:, :, D:D + 1], 1.0)
        if CFG["vsb_dve"]:
            nc.vector.tensor_copy(v_sb[:, :, 0:D],
                                  pv[:].rearrange("p (h d) -> p h d", h=H))
        else:
            nc.scalar.copy(v_sb[:, :, 0:D],
                           pv[:].rearrange("p (h d) -> p h d", h=H))
        vt.append(v_sb)

    if CFG["ph3_early"] and prev is not None:
        _emit_ph3(nc, pools, wts, out, prev)

    # interleave score groups with V token-tiles (PE filler while Act exps)
    if CFG["sv_pattern"] == 0:
        for i in range(4):
            emit_scores(i, 0)
            emit_scores(i, 1)
            emit_v(i)
    elif CFG["sv_pattern"] == 1:
        for i in range(4):
            emit_scores(i, 0)
            emit_scores(i, 1)
        for i in range(4):
            emit_v(i)
    else:
        for i in range(4):
            emit_scores(i, 0)
            emit_v(i)
            emit_scores(i, 1)

    # ---- out-projection of the previous chunk (fills the PE gap while the
    # Act engine finishes the exps this chunk's PV needs) ----
    if not CFG["ph3_early"] and prev is not None:
        _emit_ph3(nc, pools, wts, out, prev)

    # ---- PV + normalize + PE transpose back to e'-major ----
    ot_big = otp.tile([128, N_ETILE, CHUNK], BF16, name="ot", tag="ot")
    for wl in range(NW_CHUNK):
        k0 = wl * W
        for qh in range(2):
            tr4 = ps_t.tile([128, N_ETILE, 128], BF16, name=f"tr{wl}{qh}",
                            tag="tr")
            for ep in range(2):
                po = ps_o.tile([128, 2, 2 * (D + 1)], F32,
                               name=f"po{wl}{qh}{ep}", tag="po")
                for ei in range(2):
                    et = ep * 2 + ei
                    for sub in range(2):
                        h = 2 * et + sub
                        for kh in range(2):
                            nc.tensor.matmul(
                                po[:, ei, sub * (D + 1):(sub + 1) * (D + 1)],
                                p[(wl, et, sub)][:, kh, qh * 128:(qh + 1) * 128],
                                vt[wl * 2 + kh][:, h:h + 1, :],
                                start=(kh == 0), stop=(kh == 1))
                for ei in range(2):
                    et = ep * 2 + ei
                    rec = attn.tile([128, 2], F32, name=f"rc{wl}{qh}{et}",
                                    tag="rec")
                    nc.vector.reciprocal(rec[:, 0:1], po[:, ei, D:D + 1])
                    nc.vector.reciprocal(rec[:, 1:2], po[:, ei, 2 * D + 1:2 * D + 2])
                    o_sb = attn.tile([128, 128], BF16, name=f"ob{wl}{qh}{et}",
                                     tag=f"osb{et}")
                    # head 2*et (cols 0:64) on DVE, head 2*et+1 on Act
                    nc.vector.tensor_scalar_mul(
                        o_sb[:, 0:D], po[:, ei, 0:D], rec[:, 0:1])
                    if CFG["norm_split"]:
                        nc.scalar.mul(
                            o_sb[:, D:2 * D], po[:, ei, D + 1:2 * D + 1], rec[:, 1:2])
                    else:
                        nc.vector.tensor_scalar_mul(
                            o_sb[:, D:2 * D], po[:, ei, D + 1:2 * D + 1], rec[:, 1:2])
                    nc.tensor.transpose(tr4[:, et, :], o_sb[:], identb[:])
            nc.scalar.copy(ot_big[:, :, k0 + qh * 128:k0 + (qh + 1) * 128],
                           tr4[:])
    return (ot_big, s0)


def _body(tc, nc, xt, wq, wk, wv, wo, bq, out, niter):
    from contextlib import ExitStack
    ctx = ExitStack()
    with ctx:
        consts = ctx.enter_context(tc.tile_pool(name="consts", bufs=1))
        xpool = ctx.enter_context(tc.tile_pool(name="xpool", bufs=CFG["xpool_bufs"]))
        qkv = ctx.enter_context(tc.tile_pool(name="qkv", bufs=CFG.get("qkv_bufs", 2)))
        attn = ctx.enter_context(tc.tile_pool(name="attn", bufs=CFG["attn_bufs"]))
        otp = ctx.enter_context(tc.tile_pool(name="otp", bufs=CFG.get("otp_bufs", 2)))
        fo = ctx.enter_context(tc.tile_pool(name="fo", bufs=3))
        ps_big = ctx.enter_context(tc.tile_pool(name="ps_big", bufs=CFG["ps_big_bufs"], space="PSUM"))
        ps_s = ctx.enter_context(tc.tile_pool(name="ps_s", bufs=CFG["ps_s_bufs"], space="PSUM"))
        ps_o = ctx.enter_context(tc.tile_pool(name="ps_o", bufs=2, space="PSUM"))
        ps_t = ctx.enter_context(tc.tile_pool(name="ps_t", bufs=CFG["ps_t_bufs"], space="PSUM"))

        from concourse.masks import make_identity
        identb = consts.tile([128, 128], BF16, name="identb")
        make_identity(nc, identb[:])
        wq_t = [consts.tile([128, EG], BF16, name=f"wq{k}") for k in range(KT_E)]
        wk_t = [consts.tile([128, EG], BF16, name=f"wk{k}") for k in range(KT_E)]
        wv_t = [consts.tile([128, EG], BF16, name=f"wv{k}") for k in range(KT_E)]
        wo_t = [consts.tile([128, E], BF16, name=f"wo{k}") for k in range(N_ETILE)]
        bq_t = [consts.tile([128, 1], F32, name=f"bq{k}") for k in range(N_ETILE)]
        for k in range(KT_E):
            nc.gpsimd.dma_start(wq_t[k][:], wq[k * 128:(k + 1) * 128, :])
            nc.gpsimd.dma_start(wk_t[k][:], wk[k * 128:(k + 1) * 128, :])
            nc.gpsimd.dma_start(wv_t[k][:], wv[k * 128:(k + 1) * 128, :])
        for k in range(N_ETILE):
            nc.gpsimd.dma_start(wo_t[k][:], wo[k * 128:(k + 1) * 128, :])
            nc.gpsimd.dma_start(bq_t[k][:], bq[k * 128:(k + 1) * 128, :])

        pools = (consts, xpool, qkv, attn, otp, fo, ps_big, ps_s, ps_o, ps_t)
        wts = (wq_t, wk_t, wv_t, wo_t, bq_t, identb)

        def emit_all():
            prev = None
            for c in range(NCHUNK):
                prev = _chunk_body(tc, nc, pools, wts, xt, out, c, prev)
            _emit_ph3(nc, pools, wts, out, prev)

        if niter:
            with tc.For_i(0, niter, 1) as _i:
                emit_all()
        elif CFG.get("unroll", 1) > 1:   # sim-only: python-unrolled passes
            for _ in range(CFG["unroll"]):
                emit_all()
        else:
            emit_all()


def _in_maps(x, Wq, bq, Wk, Wv, Wo):
    bf = ml_dtypes.bfloat16
    sc = np.float32(1.0 / np.sqrt(D))
    in_maps = []
    for core in range(8):
        b, g = core // 2, core % 2
        gs = slice(g * EG, (g + 1) * EG)
        in_maps.append({
            "xt": np.ascontiguousarray(x[b].T).astype(bf),
            "wq": (Wq[:, gs] * sc).astype(bf),
            "wk": np.ascontiguousarray(Wk[:, gs]).astype(bf),
            "wv": np.ascontiguousarray(Wv[:, gs]).astype(bf),
            "wo": np.ascontiguousarray(Wo[gs, :]).astype(bf),
            "bq": (bq[gs] * sc).astype(np.float32).reshape(EG, 1),
        })
    return in_maps


def kernel(x, Wq, bq, Wk, bk, Wv, bv, Wo, bo):
    x, Wq, bq = np.asarray(x), np.asarray(Wq), np.asarray(bq)
    Wk, Wv, Wo = np.asarray(Wk), np.asarray(Wv), np.asarray(Wo)
    bv, bo = np.asarray(bv), np.asarray(bo)

    if "nc" not in _NC_CACHE:
        _NC_CACHE["nc"] = _build_nc()
    nc = _NC_CACHE["nc"]

    res = run_bass_kernel_spmd(nc, _in_maps(x, Wq, bq, Wk, Wv, Wo),
                               core_ids=list(range(8)))
    const_row = (bv.astype(np.float64) @ Wo.astype(np.float64)
                 + bo.astype(np.float64)).astype(np.float32)
    out = np.empty((B, S, E), np.float32)
    for b in range(B):
        out[b] = (res.results[2 * b]["out"] + res.results[2 * b + 1]["out"]
                  + const_row)
    return out



# revision 3
# speedup vs baseline: 1.0381x; 1.0381x over previous
"""Longformer block-diagonal self-attention on 8 Trainium2 NeuronCores, v3.

Sharding: core = batch*2 + seq_half.  Each core handles HALF the sequence
(2048 tokens = 8 windows, clean split since windows are non-overlapping) of
one batch, with ALL 16 heads.  Per-core output is the COMPLETE out-projection
for its token range (no partial sums across cores); host adds the constant
row (bv @ Wo + bo) and concatenates.  Output is stored bf16 (harness
tolerance 2e-2 >> bf16 rounding) which halves store traffic; inputs per core
also halve vs v2's head-split.  Chip-level HBM traffic drops ~3.5x.

Attention math is v2's design: S^T = K_w^T Q_w (key-major scores), exp on
Act, ones-column in V carries softmax denominators through the PV matmul,
1/sum applied token-major post-PV, PE transposes restore feature-major O.
bk drops out of softmax; bv contributes only the constant row -> host;
1/sqrt(head_dim) folded into Wq/bq on host.
"""

import numpy as np
import ml_dtypes

import concourse.bass as bass
import concourse.tile as tile
from concourse import bacc, mybir
from concourse.bass_utils import run_bass_kernel_spmd

F32 = mybir.dt.float32
BF16 = mybir.dt.bfloat16

B, S, E = 4, 4096, 1024
SC = S // 2                   # per-core tokens (seq half)
H, D, W = 16, 64, 256         # per-core heads (all), head dim, window
EG = E                        # full embedding per core
CHUNK = 512
NCHUNK = SC // CHUNK          # 4
NW_CHUNK = CHUNK // W         # windows per chunk (2)
KT_E = E // 128               # contraction tiles over E (8)
N_ETILE = EG // 128           # e'-tiles (8); head pair per tile

_NC_CACHE = {}

CFG = {
    "fsb_dve": True,    # f_sb out-proj copies on DVE (else Act)
    "vsb_dve": False,   # v_sb copies on DVE (else Act)
    "ps_s_bufs": 2,
    "ps_t_bufs": 2,
    "ps_big_bufs": 2,
    "ph3_early": True,  # emit prev-chunk out-proj right after QK
    "sv_pattern": 0,
    "store_sp": True,   # out-store DMAs on SP queue (else gpsimd/Pool)
    "exp_merge": True,  # one exp per (et,sub) over both kh
    "merge_recip": True,  # one strided reciprocal for both heads of a pair
    "score_pair": True,  # adjacent sub0/sub1 score MMs (row-group overlap)
    "tr_batch": True,   # emit the 8 PE transposes of a (wl,qh) back-to-back
    "tr_defer": True,   # pipeline each group's transposes one group later
    "norm_split": False,  # head-B normalize muls on Act instead of DVE
    "attn_bufs": 2,
    "xpool_bufs": 3,
    "xc_merge": False,
}


def _build_nc(niter=0):
    nc = bacc.Bacc("TRN2", target_bir_lowering=False, debug=False, num_devices=8)
    if CFG.get("xt_block", True):
        # blocked layout: chunk-major [(c, k), 128, CHUNK] so each chunk's
        # 8 x-tiles are one contiguous 1MB region in DRAM
        xt = nc.dram_tensor("xt", [NCHUNK * KT_E * 128, CHUNK], BF16,
                            kind="ExternalInput").ap()
    else:
        xt = nc.dram_tensor("xt", [E, SC], BF16, kind="ExternalInput").ap()
    wq = nc.dram_tensor("wq", [E, EG], BF16, kind="ExternalInput").ap()
    wk = nc.dram_tensor("wk", [E, EG], BF16, kind="ExternalInput").ap()
    wv = nc.dram_tensor("wv", [E, EG], BF16, kind="ExternalInput").ap()
    wo = nc.dram_tensor("wo", [EG, E], BF16, kind="ExternalInput").ap()
    bq = nc.dram_tensor("bq", [EG, 1], F32, kind="ExternalInput").ap()
    out = nc.dram_tensor("out", [SC, E], BF16, kind="ExternalOutput").ap()

    with tile.TileContext(nc) as tc:
        _body(tc, nc, xt, wq, wk, wv, wo, bq, out, niter)
    nc.compile()
    return nc


def _flush_tr(nc, identb, pend):
    tr4_, o_tiles_, dst = pend
    for et_, o_sb_ in o_tiles_:
        nc.tensor.transpose(tr4_[:, et_, :], o_sb_[:], identb[:])
    nc.scalar.copy(dst, tr4_[:])


def _emit_ph3(nc, pools, wts, out, prev):
    consts, xpool, qkv, attn, otp, fo, ps_big, ps_s, ps_o, ps_t = pools
    wq_t, wk_t, wv_t, wo_t, bq_t, identb = wts
    ot_big, s0, pending = prev
    if pending is not None:
        _flush_tr(nc, identb, pending)
        prev[2] = None
    for t in range(CHUNK // 128):
        f_sb = fo.tile([128, E], BF16, name=f"f{t}", tag="fout")
        for eh in range(2):
            pf = ps_big.tile([128, 512], F32, name=f"pf{t}_{eh}", tag="big")
            for k4 in range(N_ETILE):
                nc.tensor.matmul(pf[:],
                                 ot_big[:, k4, t * 128:(t + 1) * 128],
                                 wo_t[k4][:, eh * 512:(eh + 1) * 512],
                                 start=(k4 == 0), stop=(k4 == N_ETILE - 1))
            if CFG["fsb_dve"]:
                nc.vector.tensor_copy(f_sb[:, eh * 512:(eh + 1) * 512], pf[:])
            else:
                nc.scalar.copy(f_sb[:, eh * 512:(eh + 1) * 512], pf[:])
        eng = nc.sync if CFG["store_sp"] else nc.gpsimd
        eng.dma_start(out[s0 + t * 128:s0 + (t + 1) * 128, :], f_sb[:])


def _chunk_body(tc, nc, pools, wts, xt, out, c, prev):
    consts, xpool, qkv, attn, otp, fo, ps_big, ps_s, ps_o, ps_t = pools
    wq_t, wk_t, wv_t, wo_t, bq_t, identb = wts
    s0 = c * CHUNK

    if CFG.get("xc_merge", True):
        xcb = xpool.tile([128, KT_E, CHUNK], BF16, name="xc", tag="xc")
        if CFG.get("xt_block", True):
            src = xt[c * KT_E * 128:(c + 1) * KT_E * 128, :]
        else:
            src = xt[:, s0:s0 + CHUNK]
        nc.gpsimd.dma_start(xcb[:], src.rearrange("(k p) j -> p k j", p=128))
        xc = [xcb[:, k, :] for k in range(KT_E)]
    else:
        xc = []
        for k in range(KT_E):
            t = xpool.tile([128, CHUNK], BF16, name=f"xc{k}", tag=f"xc{k}")
            if CFG.get("xt_block", True):
                blk = c * KT_E + k
                nc.gpsimd.dma_start(t[:], xt[blk * 128:(blk + 1) * 128, :])
            else:
                nc.gpsimd.dma_start(
                    t[:], xt[k * 128:(k + 1) * 128, s0:s0 + CHUNK])
            xc.append(t)

    # ---- Q^T, K^T (e'-major) ----
    qt, kt = [], []
    for t in range(N_ETILE):
        pq = ps_big.tile([128, CHUNK], F32, name=f"pq{t}", tag="big")
        for k in range(KT_E):
            nc.tensor.matmul(pq[:], wq_t[k][:, t * 128:(t + 1) * 128],
                             xc[k][:], start=(k == 0), stop=(k == KT_E - 1))
        q_sb = qkv.tile([128, CHUNK], BF16, name=f"qt{t}", tag=f"qt{t}")
        nc.vector.tensor_scalar_add(q_sb[:], pq[:], bq_t[t][:])
        qt.append(q_sb)

        pk = ps_big.tile([128, CHUNK], F32, name=f"pk{t}", tag="big")
        for k in range(KT_E):
            nc.tensor.matmul(pk[:], wk_t[k][:, t * 128:(t + 1) * 128],
                             xc[k][:], start=(k == 0), stop=(k == KT_E - 1))
        k_sb = qkv.tile([128, CHUNK], BF16, name=f"kt{t}", tag=f"kt{t}")
        nc.scalar.copy(k_sb[:], pk[:])
        kt.append(k_sb)

    # ---- scores^T + exp, interleaved with V projection so the Act-engine
    # exp burst hides under PE matmuls ----
    p = {}
    vt = []

    def emit_scores(et, sub):
        prow = sub * 64
        for wl in range(NW_CHUNK):
            k0 = wl * W
            ps = ps_s.tile([128, 2, W], F32, name=f"s{et}{sub}{wl}", tag="sc")
            for kh in range(2):
                nc.tensor.matmul(
                    ps[:, kh, :],
                    kt[et][prow:prow + 64, k0 + kh * 128:k0 + (kh + 1) * 128],
                    qt[et][prow:prow + 64, k0:k0 + W],
                    start=True, stop=True)
            pt = attn.tile([128, 2, W], BF16, name=f"p{et}{sub}",
                           tag=f"p{et}{sub}")
            if CFG["exp_merge"]:
                nc.scalar.activation(pt[:], ps[:],
                                     mybir.ActivationFunctionType.Exp)
            else:
                for kh in range(2):
                    nc.scalar.activation(pt[:, kh, :], ps[:, kh, :],
                                         mybir.ActivationFunctionType.Exp)
            p[(wl, et, sub)] = pt

    def emit_scores_paired(et):
        # sub=0 MMs use contraction partitions 0-63 (PE row groups 0-1),
        # sub=1 partitions 64-127 (row groups 2-3).  Emitting each (wl, kh)
        # as an adjacent sub0/sub1 pair lets the PE run them concurrently.
        for wl in range(NW_CHUNK):
            k0 = wl * W
            pss = [ps_s.tile([128, 2, W], F32, name=f"s{et}{sub}{wl}",
                             tag="sc") for sub in range(2)]
            for kh in range(2):
                for sub in range(2):
                    prow = sub * 64
                    nc.tensor.matmul(
                        pss[sub][:, kh, :],
                        kt[et][prow:prow + 64, k0 + kh * 128:k0 + (kh + 1) * 128],
                        qt[et][prow:prow + 64, k0:k0 + W],
                        start=True, stop=True)
            for sub in range(2):
                pt = attn.tile([128, 2, W], BF16, name=f"p{et}{sub}",
                               tag=f"p{et}{sub}")
                nc.scalar.activation(pt[:], pss[sub][:],
                                     mybir.ActivationFunctionType.Exp)
                p[(wl, et, sub)] = pt

    def emit_v(t):
        # V token-major: stationary x-slice, moving Wv; EG=1024 needs two
        # 512-col PSUM halves (heads 0-7 | 8-15)
        v_sb = qkv.tile([128, H, D + 1], BF16, name=f"vt{t}", tag=f"vt{t}")
        nc.vector.memset(v_sb[:, :, D:D + 1], 1.0)
        for vh in range(2):
            pv = ps_big.tile([128, 512], F32, name=f"pv{t}_{vh}", tag="big")
            for k in range(KT_E):
                nc.tensor.matmul(pv[:], xc[k][:, t * 128:(t + 1) * 128],
                                 wv_t[k][:, vh * 512:(vh + 1) * 512],
                                 start=(k == 0), stop=(k == KT_E - 1))
            dst = v_sb[:, vh * (H // 2):(vh + 1) * (H // 2), 0:D]
            src = pv[:].rearrange("p (h d) -> p h d", h=H // 2)
            if CFG["vsb_dve"]:
                nc.vector.tensor_copy(dst, src)
            else:
                nc.scalar.copy(dst, src)
        vt.append(v_sb)

    if CFG["ph3_early"] and prev is not None:
        _emit_ph3(nc, pools, wts, out, prev)

    # interleave score groups (8 et) with V token-tiles (4)
    if CFG["score_pair"]:
        for i in range(4):
            emit_scores_paired(2 * i)
            emit_scores_paired(2 * i + 1)
            emit_v(i)
    elif CFG["sv_pattern"] == 0:
        for i in range(4):
            emit_scores(2 * i, 0)
            emit_scores(2 * i, 1)
            emit_scores(2 * i + 1, 0)
            emit_scores(2 * i + 1, 1)
            emit_v(i)
    else:
        for i in range(8):
            emit_scores(i, 0)
            emit_scores(i, 1)
        for i in range(4):
            emit_v(i)

    if not CFG["ph3_early"] and prev is not None:
        _emit_ph3(nc, pools, wts, out, prev)

    # ---- PV + normalize + PE transpose back to e'-major ----
    # tr_defer pipelines each group's transposes one group later, so the PE
    # runs group g+1's PV matmuls while the DVE/Act normalize chain of group
    # g completes (PE is in-order; without this the transposes stall it).
    ot_big = otp.tile([128, N_ETILE, CHUNK], BF16, name="ot", tag="ot")
    pending = None   # (tr4, o_tiles, ot_dst)
    for wl in range(NW_CHUNK):
        k0 = wl * W
        for qh in range(2):
            tr4 = ps_t.tile([128, N_ETILE, 128], BF16, name=f"tr{wl}{qh}",
                            tag="tr")
            o_tiles = []
            for ep in range(N_ETILE // 2):
                po = ps_o.tile([128, 2, 2 * (D + 1)], F32,
                               name=f"po{wl}{qh}{ep}", tag="po")
                for ei in range(2):
                    et = ep * 2 + ei
                    for sub in range(2):
                        h = 2 * et + sub
                        for kh in range(2):
                            nc.tensor.matmul(
                                po[:, ei, sub * (D + 1):(sub + 1) * (D + 1)],
                                p[(wl, et, sub)][:, kh, qh * 128:(qh + 1) * 128],
                                vt[wl * 2 + kh][:, h:h + 1, :],
                                start=(kh == 0), stop=(kh == 1))
                for ei in range(2):
                    et = ep * 2 + ei
                    rec = attn.tile([128, 2], F32, name=f"rc{wl}{qh}{et}",
                                    tag="rec")
                    if CFG["merge_recip"]:
                        nc.vector.reciprocal(rec[:], po[:, ei, D::D + 1])
                    else:
                        nc.vector.reciprocal(rec[:, 0:1], po[:, ei, D:D + 1])
                        nc.vector.reciprocal(rec[:, 1:2],
                                             po[:, ei, 2 * D + 1:2 * D + 2])
                    o_sb = attn.tile([128, 128], BF16, name=f"ob{wl}{qh}{et}",
                                     tag=f"osb{et}")
                    nc.vector.tensor_scalar_mul(
                        o_sb[:, 0:D], po[:, ei, 0:D], rec[:, 0:1])
                    if CFG["norm_split"]:
                        nc.scalar.mul(
                            o_sb[:, D:2 * D], po[:, ei, D + 1:2 * D + 1],
                            rec[:, 1:2])
                    else:
                        nc.vector.tensor_scalar_mul(
                            o_sb[:, D:2 * D], po[:, ei, D + 1:2 * D + 1],
                            rec[:, 1:2])
                    if CFG["tr_batch"]:
                        o_tiles.append((et, o_sb))
                    else:
                        nc.tensor.transpose(tr4[:, et, :], o_sb[:], identb[:])
            dst = ot_big[:, :, k0 + qh * 128:k0 + (qh + 1) * 128]
            if CFG["tr_defer"]:
                if pending is not None:
                    _flush_tr(nc, identb, pending)
                pending = (tr4, o_tiles, dst)
            else:
                for et, o_sb in o_tiles:
                    nc.tensor.transpose(tr4[:, et, :], o_sb[:], identb[:])
                nc.scalar.copy(dst, tr4[:])
    return [ot_big, s0, pending]


def _body(tc, nc, xt, wq, wk, wv, wo, bq, out, niter):
    from contextlib import ExitStack
    ctx = ExitStack()
    with ctx:
        consts = ctx.enter_context(tc.tile_pool(name="consts", bufs=1))
        xpool = ctx.enter_context(tc.tile_pool(name="xpool", bufs=CFG["xpool_bufs"]))
        qkv = ctx.enter_context(tc.tile_pool(name="qkv", bufs=CFG.get("qkv_bufs", 2)))
        attn = ctx.enter_context(tc.tile_pool(name="attn", bufs=CFG["attn_bufs"]))
        otp = ctx.enter_context(tc.tile_pool(name="otp", bufs=CFG.get("otp_bufs", 2)))
        fo = ctx.enter_context(tc.tile_pool(name="fo", bufs=3))
        ps_big = ctx.enter_context(tc.tile_pool(name="ps_big", bufs=CFG["ps_big_bufs"], space="PSUM"))
        ps_s = ctx.enter_context(tc.tile_pool(name="ps_s", bufs=CFG["ps_s_bufs"], space="PSUM"))
        ps_o = ctx.enter_context(tc.tile_pool(name="ps_o", bufs=2, space="PSUM"))
        ps_t = ctx.enter_context(tc.tile_pool(name="ps_t", bufs=CFG["ps_t_bufs"], space="PSUM"))

        from concourse.masks import make_identity
        identb = consts.tile([128, 128], BF16, name="identb")
        make_identity(nc, identb[:])
        wq_t = [consts.tile([128, EG], BF16, name=f"wq{k}") for k in range(KT_E)]
        wk_t = [consts.tile([128, EG], BF16, name=f"wk{k}") for k in range(KT_E)]
        wv_t = [consts.tile([128, EG], BF16, name=f"wv{k}") for k in range(KT_E)]
        wo_t = [consts.tile([128, E], BF16, name=f"wo{k}") for k in range(N_ETILE)]
        bq_t = [consts.tile([128, 1], F32, name=f"bq{k}") for k in range(N_ETILE)]
        for k in range(KT_E):
            nc.gpsimd.dma_start(wq_t[k][:], wq[k * 128:(k + 1) * 128, :])
            nc.gpsimd.dma_start(wk_t[k][:], wk[k * 128:(k + 1) * 128, :])
            nc.gpsimd.dma_start(wv_t[k][:], wv[k * 128:(k + 1) * 128, :])
        for k in range(N_ETILE):
            nc.gpsimd.dma_start(wo_t[k][:], wo[k * 128:(k + 1) * 128, :])
            nc.gpsimd.dma_start(bq_t[k][:], bq[k * 128:(k + 1) * 128, :])

        pools = (consts, xpool, qkv, attn, otp, fo, ps_big, ps_s, ps_o, ps_t)
        wts = (wq_t, wk_t, wv_t, wo_t, bq_t, identb)

        def emit_all():
            prev = None
            for c in range(NCHUNK):
                prev = _chunk_body(tc, nc, pools, wts, xt, out, c, prev)
            _emit_ph3(nc, pools, wts, out, prev)

        if niter:
            with tc.For_i(0, niter, 1,
                          staggered_reset=CFG.get("staggered", False)) as _i:
                emit_all()
        elif CFG.get("unroll", 1) > 1:   # sim-only: python-unrolled passes
            for _ in range(CFG["unroll"]):
                emit_all()
        else:
            emit_all()


def _in_maps(x, Wq, bq, Wk, Wv, Wo):
    bf = ml_dtypes.bfloat16
    sc = np.float32(1.0 / np.sqrt(D))
    wqs = (np.asarray(Wq) * sc).astype(bf)
    wks = np.asarray(Wk).astype(bf)
    wvs = np.asarray(Wv).astype(bf)
    wos = np.asarray(Wo).astype(bf)
    bqs = (np.asarray(bq) * sc).astype(np.float32).reshape(EG, 1)
    in_maps = []
    for core in range(8):
        b, hf = core // 2, core % 2
        xth = np.ascontiguousarray(x[b, hf * SC:(hf + 1) * SC].T).astype(bf)
        if CFG.get("xt_block", True):
            # [E, SC] -> [(c k) p, j]: chunk-major contiguous 1MB blocks
            xth = np.ascontiguousarray(
                xth.reshape(KT_E, 128, NCHUNK, CHUNK)
                   .transpose(2, 0, 1, 3)
                   .reshape(NCHUNK * KT_E * 128, CHUNK))
        in_maps.append({
            "xt": xth,
            "wq": wqs, "wk": wks, "wv": wvs, "wo": wos, "bq": bqs,
        })
    return in_maps


def kernel(x, Wq, bq, Wk, bk, Wv, bv, Wo, bo):
    x, Wq, bq = np.asarray(x), np.asarray(Wq), np.asarray(bq)
    Wk, Wv, Wo = np.asarray(Wk), np.asarray(Wv), np.asarray(Wo)
    bv, bo = np.asarray(bv), np.asarray(bo)

    if "nc" not in _NC_CACHE:
        _NC_CACHE["nc"] = _build_nc()
    nc = _NC_CACHE["nc"]

    res = run_bass_kernel_spmd(nc, _in_maps(x, Wq, bq, Wk, Wv, Wo),
                               core_ids=list(range(8)))
    const_row = (bv.astype(np.float64) @ Wo.astype(np.float64)
                 + bo.astype(np.float64)).astype(np.float32)
    out = np.empty((B, S, E), np.float32)
    for core in range(8):
        b, hf = core // 2, core % 2
        out[b, hf * SC:(hf + 1) * SC] = (
            np.asarray(res.results[core]["out"]).astype(np.float32) + const_row)
    return out


# revision 6
# speedup vs baseline: 1.0632x; 1.0242x over previous
"""Longformer block-diagonal self-attention on 8 Trainium2 NeuronCores, v3.

Sharding: core = batch*2 + seq_half.  Each core handles HALF the sequence
(2048 tokens = 8 windows, clean split since windows are non-overlapping) of
one batch, with ALL 16 heads.  Per-core output is the COMPLETE out-projection
for its token range (no partial sums across cores); host adds the constant
row (bv @ Wo + bo) and concatenates.  Output is stored bf16 (harness
tolerance 2e-2 >> bf16 rounding) which halves store traffic; inputs per core
also halve vs v2's head-split.  Chip-level HBM traffic drops ~3.5x.

Attention math is v2's design: S^T = K_w^T Q_w (key-major scores), exp on
Act, ones-column in V carries softmax denominators through the PV matmul,
1/sum applied token-major post-PV, PE transposes restore feature-major O.
bk drops out of softmax; bv contributes only the constant row -> host;
1/sqrt(head_dim) folded into Wq/bq on host.
"""

import numpy as np
import ml_dtypes

import concourse.bass as bass
import concourse.tile as tile
from concourse import bacc, mybir
from concourse.bass_utils import run_bass_kernel_spmd

F32 = mybir.dt.float32
BF16 = mybir.dt.bfloat16

B, S, E = 4, 4096, 1024
SC = S // 2                   # per-core tokens (seq half)
H, D, W = 16, 64, 256         # per-core heads (all), head dim, window
EG = E                        # full embedding per core
CHUNK = 512
NCHUNK = SC // CHUNK          # 4
NW_CHUNK = CHUNK // W         # windows per chunk (2)
KT_E = E // 128               # contraction tiles over E (8)
N_ETILE = EG // 128           # e'-tiles (8); head pair per tile

_NC_CACHE = {}

CFG = {
    "fsb_dve": False,    # f_sb out-proj copies on DVE (else Act)
    "vsb_dve": True,   # v_sb copies on DVE (else Act)
    "ps_s_bufs": 2,
    "ps_t_bufs": 2,
    "ps_big_bufs": 2,
    "ph3_early": True,  # emit prev-chunk out-proj right after QK
    "sv_pattern": 0,
    "store_sp": True,   # out-store DMAs on SP queue (else gpsimd/Pool)
    "exp_merge": True,  # one exp per (et,sub) over both kh
    "merge_recip": True,  # one strided reciprocal for both heads of a pair
    "score_pair": True,  # adjacent sub0/sub1 score MMs (row-group overlap)
    "tr_batch": True,   # emit the 8 PE transposes of a (wl,qh) back-to-back
    "tr_defer": True,   # pipeline each group's transposes one group later
    "norm_split": False,  # head-B normalize muls on Act instead of DVE
    "attn_bufs": 2,
    "xpool_bufs": 3,
    "xc_merge": False,
}


def _build_nc(niter=0):
    nc = bacc.Bacc("TRN2", target_bir_lowering=False, debug=False, num_devices=8)
    if CFG.get("xt_block", True):
        # blocked layout: chunk-major [(c, k), 128, CHUNK] so each chunk's
        # 8 x-tiles are one contiguous 1MB region in DRAM
        xt = nc.dram_tensor("xt", [NCHUNK * KT_E * 128, CHUNK], BF16,
                            kind="ExternalInput").ap()
    else:
        xt = nc.dram_tensor("xt", [E, SC], BF16, kind="ExternalInput").ap()
    wq = nc.dram_tensor("wq", [E, EG], BF16, kind="ExternalInput").ap()
    wk = nc.dram_tensor("wk", [E, EG], BF16, kind="ExternalInput").ap()
    wv = nc.dram_tensor("wv", [E, EG], BF16, kind="ExternalInput").ap()
    wo = nc.dram_tensor("wo", [EG, E], BF16, kind="ExternalInput").ap()
    bq = nc.dram_tensor("bq", [EG, 1], F32, kind="ExternalInput").ap()
    out = nc.dram_tensor("out", [SC, E], BF16, kind="ExternalOutput").ap()

    with tile.TileContext(nc) as tc:
        _body(tc, nc, xt, wq, wk, wv, wo, bq, out, niter)
    nc.compile()
    return nc


def _flush_tr(nc, identb, pend):
    tr4_, o_tiles_, dst = pend
    for et_, o_sb_ in o_tiles_:
        nc.tensor.transpose(tr4_[:, et_, :], o_sb_[:], identb[:])
    nc.scalar.copy(dst, tr4_[:])


def _emit_ph3(nc, pools, wts, out, prev):
    consts, xpool, qkv, attn, otp, fo, ps_big, ps_s, ps_o, ps_t = pools
    wq_t, wk_t, wv_t, wo_t, bq_t, identb = wts
    ot_big, s0, pending = prev
    if pending is not None:
        _flush_tr(nc, identb, pending)
        prev[2] = None
    for t in range(CHUNK // 128):
        f_sb = fo.tile([128, E], BF16, name=f"f{t}", tag="fout")
        for eh in range(2):
            pf = ps_big.tile([128, 512], F32, name=f"pf{t}_{eh}", tag="big")
            for k4 in range(N_ETILE):
                nc.tensor.matmul(pf[:],
                                 ot_big[:, k4, t * 128:(t + 1) * 128],
                                 wo_t[k4][:, eh * 512:(eh + 1) * 512],
                                 start=(k4 == 0), stop=(k4 == N_ETILE - 1))
            if CFG["fsb_dve"]:
                nc.vector.tensor_copy(f_sb[:, eh * 512:(eh + 1) * 512], pf[:])
            else:
                nc.scalar.copy(f_sb[:, eh * 512:(eh + 1) * 512], pf[:])
        eng = nc.sync if CFG["store_sp"] else nc.gpsimd
        eng.dma_start(out[s0 + t * 128:s0 + (t + 1) * 128, :], f_sb[:])


def _chunk_body(tc, nc, pools, wts, xt, out, c, prev):
    consts, xpool, qkv, attn, otp, fo, ps_big, ps_s, ps_o, ps_t = pools
    wq_t, wk_t, wv_t, wo_t, bq_t, identb = wts
    s0 = c * CHUNK

    if CFG.get("xc_merge", True):
        xcb = xpool.tile([128, KT_E, CHUNK], BF16, name="xc", tag="xc")
        if CFG.get("xt_block", True):
            src = xt[c * KT_E * 128:(c + 1) * KT_E * 128, :]
        else:
            src = xt[:, s0:s0 + CHUNK]
        nc.gpsimd.dma_start(xcb[:], src.rearrange("(k p) j -> p k j", p=128))
        xc = [xcb[:, k, :] for k in range(KT_E)]
    else:
        xc = []
        for k in range(KT_E):
            t = xpool.tile([128, CHUNK], BF16, name=f"xc{k}", tag=f"xc{k}")
            if CFG.get("xt_block", True):
                blk = c * KT_E + k
                nc.gpsimd.dma_start(t[:], xt[blk * 128:(blk + 1) * 128, :])
            else:
                nc.gpsimd.dma_start(
                    t[:], xt[k * 128:(k + 1) * 128, s0:s0 + CHUNK])
            xc.append(t)

    # ---- Q^T, K^T (e'-major) ----
    qt, kt = [], []
    for t in range(N_ETILE):
        pq = ps_big.tile([128, CHUNK], F32, name=f"pq{t}", tag="big")
        for k in range(KT_E):
            nc.tensor.matmul(pq[:], wq_t[k][:, t * 128:(t + 1) * 128],
                             xc[k][:], start=(k == 0), stop=(k == KT_E - 1))
        q_sb = qkv.tile([128, CHUNK], BF16, name=f"qt{t}", tag=f"qt{t}")
        nc.vector.tensor_scalar_add(q_sb[:], pq[:], bq_t[t][:])
        qt.append(q_sb)

        pk = ps_big.tile([128, CHUNK], F32, name=f"pk{t}", tag="big")
        for k in range(KT_E):
            nc.tensor.matmul(pk[:], wk_t[k][:, t * 128:(t + 1) * 128],
                             xc[k][:], start=(k == 0), stop=(k == KT_E - 1))
        k_sb = qkv.tile([128, CHUNK], BF16, name=f"kt{t}", tag=f"kt{t}")
        nc.scalar.copy(k_sb[:], pk[:])
        kt.append(k_sb)

    # ---- scores^T + exp, interleaved with V projection so the Act-engine
    # exp burst hides under PE matmuls ----
    p = {}
    vt = []

    def emit_scores(et, sub):
        prow = sub * 64
        for wl in range(NW_CHUNK):
            k0 = wl * W
            ps = ps_s.tile([128, 2, W], F32, name=f"s{et}{sub}{wl}", tag="sc")
            for kh in range(2):
                nc.tensor.matmul(
                    ps[:, kh, :],
                    kt[et][prow:prow + 64, k0 + kh * 128:k0 + (kh + 1) * 128],
                    qt[et][prow:prow + 64, k0:k0 + W],
                    start=True, stop=True)
            pt = attn.tile([128, 2, W], BF16, name=f"p{et}{sub}",
                           tag=f"p{et}{sub}")
            if CFG["exp_merge"]:
                nc.scalar.activation(pt[:], ps[:],
                                     mybir.ActivationFunctionType.Exp)
            else:
                for kh in range(2):
                    nc.scalar.activation(pt[:, kh, :], ps[:, kh, :],
                                         mybir.ActivationFunctionType.Exp)
            p[(wl, et, sub)] = pt

    def emit_scores_paired(et):
        # sub=0 MMs use contraction partitions 0-63 (PE row groups 0-1),
        # sub=1 partitions 64-127 (row groups 2-3).  Emitting each (wl, kh)
        # as an adjacent sub0/sub1 pair lets the PE run them concurrently.
        for wl in range(NW_CHUNK):
            k0 = wl * W
            pss = [ps_s.tile([128, 2, W], F32, name=f"s{et}{sub}{wl}",
                             tag="sc") for sub in range(2)]
            for kh in range(2):
                for sub in range(2):
                    prow = sub * 64
                    nc.tensor.matmul(
                        pss[sub][:, kh, :],
                        kt[et][prow:prow + 64, k0 + kh * 128:k0 + (kh + 1) * 128],
                        qt[et][prow:prow + 64, k0:k0 + W],
                        start=True, stop=True)
            for sub in range(2):
                pt = attn.tile([128, 2, W], BF16, name=f"p{et}{sub}",
                               tag=f"p{et}{sub}")
                nc.scalar.activation(pt[:], pss[sub][:],
                                     mybir.ActivationFunctionType.Exp)
                p[(wl, et, sub)] = pt

    def emit_v(t):
        # V token-major: stationary x-slice, moving Wv; EG=1024 needs two
        # 512-col PSUM halves (heads 0-7 | 8-15)
        v_sb = qkv.tile([128, H, D + 1], BF16, name=f"vt{t}", tag=f"vt{t}")
        nc.vector.memset(v_sb[:, :, D:D + 1], 1.0)
        for vh in range(2):
            pv = ps_big.tile([128, 512], F32, name=f"pv{t}_{vh}", tag="big")
            for k in range(KT_E):
                nc.tensor.matmul(pv[:], xc[k][:, t * 128:(t + 1) * 128],
                                 wv_t[k][:, vh * 512:(vh + 1) * 512],
                                 start=(k == 0), stop=(k == KT_E - 1))
            dst = v_sb[:, vh * (H // 2):(vh + 1) * (H // 2), 0:D]
            src = pv[:].rearrange("p (h d) -> p h d", h=H // 2)
            if CFG["vsb_dve"]:
                nc.vector.tensor_copy(dst, src)
            else:
                nc.scalar.copy(dst, src)
        vt.append(v_sb)

    if CFG["ph3_early"] and prev is not None:
        _emit_ph3(nc, pools, wts, out, prev)

    # interleave score groups (8 et) with V token-tiles (4)
    if CFG["score_pair"]:
        for i in range(4):
            emit_scores_paired(2 * i)
            emit_scores_paired(2 * i + 1)
            emit_v(i)
    elif CFG["sv_pattern"] == 0:
        for i in range(4):
            emit_scores(2 * i, 0)
            emit_scores(2 * i, 1)
            emit_scores(2 * i + 1, 0)
            emit_scores(2 * i + 1, 1)
            emit_v(i)
    else:
        for i in range(8):
            emit_scores(i, 0)
            emit_scores(i, 1)
        for i in range(4):
            emit_v(i)

    if not CFG["ph3_early"] and prev is not None:
        _emit_ph3(nc, pools, wts, out, prev)

    # ---- PV + normalize + PE transpose back to e'-major ----
    # tr_defer pipelines each group's transposes one group later, so the PE
    # runs group g+1's PV matmuls while the DVE/Act normalize chain of group
    # g completes (PE is in-order; without this the transposes stall it).
    ot_big = otp.tile([128, N_ETILE, CHUNK], BF16, name="ot", tag="ot")
    pending = None   # (tr4, o_tiles, ot_dst)
    for wl in range(NW_CHUNK):
        k0 = wl * W
        for qh in range(2):
            tr4 = ps_t.tile([128, N_ETILE, 128], BF16, name=f"tr{wl}{qh}",
                            tag="tr")
            o_tiles = []
            for ep in range(N_ETILE // 2):
                po = ps_o.tile([128, 2, 2 * (D + 1)], F32,
                               name=f"po{wl}{qh}{ep}", tag="po")
                for ei in range(2):
                    et = ep * 2 + ei
                    for sub in range(2):
                        h = 2 * et + sub
                        for kh in range(2):
                            nc.tensor.matmul(
                                po[:, ei, sub * (D + 1):(sub + 1) * (D + 1)],
                                p[(wl, et, sub)][:, kh, qh * 128:(qh + 1) * 128],
                                vt[wl * 2 + kh][:, h:h + 1, :],
                                start=(kh == 0), stop=(kh == 1))
                if CFG.get("recip4", True):
                    # one reciprocal covers all 4 sums of the po tile
                    # (both ei, both sub): strided AP picks cols D, 2D+1
                    # within each ei block
                    rec = attn.tile([128, 2, 2], F32, name=f"rc{wl}{qh}{ep}",
                                    tag="rec")
                    nc.vector.reciprocal(rec[:], po[:, :, D::D + 1])
                    recs = [rec[:, 0, :], rec[:, 1, :]]
                else:
                    recs = []
                    for ei in range(2):
                        r = attn.tile([128, 2], F32,
                                      name=f"rc{wl}{qh}{ep}{ei}", tag="rec")
                        nc.vector.reciprocal(r[:], po[:, ei, D::D + 1])
                        recs.append(r[:])
                for ei in range(2):
                    et = ep * 2 + ei
                    rec = recs[ei]
                    o_sb = attn.tile([128, 128], BF16, name=f"ob{wl}{qh}{et}",
                                     tag=f"osb{et}")
                    nc.vector.tensor_scalar_mul(
                        o_sb[:, 0:D], po[:, ei, 0:D], rec[:, 0:1])
                    if CFG["norm_split"]:
                        nc.scalar.mul(
                            o_sb[:, D:2 * D], po[:, ei, D + 1:2 * D + 1],
                            rec[:, 1:2])
                    else:
                        nc.vector.tensor_scalar_mul(
                            o_sb[:, D:2 * D], po[:, ei, D + 1:2 * D + 1],
                            rec[:, 1:2])
                    if CFG["tr_batch"]:
                        o_tiles.append((et, o_sb))
                    else:
                        nc.tensor.transpose(tr4[:, et, :], o_sb[:], identb[:])
            dst = ot_big[:, :, k0 + qh * 128:k0 + (qh + 1) * 128]
            if CFG["tr_defer"]:
                if pending is not None:
                    _flush_tr(nc, identb, pending)
                pending = (tr4, o_tiles, dst)
            else:
                for et, o_sb in o_tiles:
                    nc.tensor.transpose(tr4[:, et, :], o_sb[:], identb[:])
                nc.scalar.copy(dst, tr4[:])
    return [ot_big, s0, pending]


def _body(tc, nc, xt, wq, wk, wv, wo, bq, out, niter):
    from contextlib import ExitStack
    ctx = ExitStack()
    with ctx:
        consts = ctx.enter_context(tc.tile_pool(name="consts", bufs=1))
        xpool = ctx.enter_context(tc.tile_pool(name="xpool", bufs=CFG["xpool_bufs"]))
        qkv = ctx.enter_context(tc.tile_pool(name="qkv", bufs=CFG.get("qkv_bufs", 2)))
        attn = ctx.enter_context(tc.tile_pool(name="attn", bufs=CFG["attn_bufs"]))
        otp = ctx.enter_context(tc.tile_pool(name="otp", bufs=CFG.get("otp_bufs", 2)))
        fo = ctx.enter_context(tc.tile_pool(name="fo", bufs=3))
        ps_big = ctx.enter_context(tc.tile_pool(name="ps_big", bufs=CFG["ps_big_bufs"], space="PSUM"))
        ps_s = ctx.enter_context(tc.tile_pool(name="ps_s", bufs=CFG["ps_s_bufs"], space="PSUM"))
        ps_o = ctx.enter_context(tc.tile_pool(name="ps_o", bufs=CFG.get("ps_o_bufs", 2), space="PSUM"))
        ps_t = ctx.enter_context(tc.tile_pool(name="ps_t", bufs=CFG["ps_t_bufs"], space="PSUM"))

        from concourse.masks import make_identity
        identb = consts.tile([128, 128], BF16, name="identb")
        make_identity(nc, identb[:])
        wq_t = [consts.tile([128, EG], BF16, name=f"wq{k}") for k in range(KT_E)]
        wk_t = [consts.tile([128, EG], BF16, name=f"wk{k}") for k in range(KT_E)]
        wv_t = [consts.tile([128, EG], BF16, name=f"wv{k}") for k in range(KT_E)]
        wo_t = [consts.tile([128, E], BF16, name=f"wo{k}") for k in range(N_ETILE)]
        bq_t = [consts.tile([128, 1], F32, name=f"bq{k}") for k in range(N_ETILE)]
        for k in range(KT_E):
            nc.gpsimd.dma_start(wq_t[k][:], wq[k * 128:(k + 1) * 128, :])
            nc.gpsimd.dma_start(wk_t[k][:], wk[k * 128:(k + 1) * 128, :])
            nc.gpsimd.dma_start(wv_t[k][:], wv[k * 128:(k + 1) * 128, :])
        for k in range(N_ETILE):
            nc.gpsimd.dma_start(wo_t[k][:], wo[k * 128:(k + 1) * 128, :])
            nc.gpsimd.dma_start(bq_t[k][:], bq[k * 128:(k + 1) * 128, :])

        pools = (consts, xpool, qkv, attn, otp, fo, ps_big, ps_s, ps_o, ps_t)
        wts = (wq_t, wk_t, wv_t, wo_t, bq_t, identb)

        def emit_all():
            prev = None
            for c in range(NCHUNK):
                prev = _chunk_body(tc, nc, pools, wts, xt, out, c, prev)
            _emit_ph3(nc, pools, wts, out, prev)

        if niter:
            hints = tuple(mybir.ALL_ENGINES) if CFG.get("loop_hints") else ()
            with tc.For_i(0, niter, 1, hint_engines=hints,
                          staggered_reset=CFG.get("staggered", False)) as _i:
                emit_all()
        elif CFG.get("unroll", 1) > 1:   # sim-only: python-unrolled passes
            for _ in range(CFG["unroll"]):
                emit_all()
        else:
            emit_all()


def _in_maps(x, Wq, bq, Wk, Wv, Wo):
    bf = ml_dtypes.bfloat16
    sc = np.float32(1.0 / np.sqrt(D))
    wqs = (np.asarray(Wq) * sc).astype(bf)
    wks = np.asarray(Wk).astype(bf)
    wvs = np.asarray(Wv).astype(bf)
    wos = np.asarray(Wo).astype(bf)
    bqs = (np.asarray(bq) * sc).astype(np.float32).reshape(EG, 1)
    in_maps = []
    for core in range(8):
        b, hf = core // 2, core % 2
        xth = np.ascontiguousarray(x[b, hf * SC:(hf + 1) * SC].T).astype(bf)
        if CFG.get("xt_block", True):
            # [E, SC] -> [(c k) p, j]: chunk-major contiguous 1MB blocks
            xth = np.ascontiguousarray(
                xth.reshape(KT_E, 128, NCHUNK, CHUNK)
                   .transpose(2, 0, 1, 3)
                   .reshape(NCHUNK * KT_E * 128, CHUNK))
        in_maps.append({
            "xt": xth,
            "wq": wqs, "wk": wks, "wv": wvs, "wo": wos, "bq": bqs,
        })
    return in_maps


def kernel(x, Wq, bq, Wk, bk, Wv, bv, Wo, bo):
    x, Wq, bq = np.asarray(x), np.asarray(Wq), np.asarray(bq)
    Wk, Wv, Wo = np.asarray(Wk), np.asarray(Wv), np.asarray(Wo)
    bv, bo = np.asarray(bv), np.asarray(bo)

    if "nc" not in _NC_CACHE:
        _NC_CACHE["nc"] = _build_nc()
    nc = _NC_CACHE["nc"]

    res = run_bass_kernel_spmd(nc, _in_maps(x, Wq, bq, Wk, Wv, Wo),
                               core_ids=list(range(8)))
    const_row = (bv.astype(np.float64) @ Wo.astype(np.float64)
                 + bo.astype(np.float64)).astype(np.float32)
    out = np.empty((B, S, E), np.float32)
    for core in range(8):
        b, hf = core // 2, core % 2
        out[b, hf * SC:(hf + 1) * SC] = (
            np.asarray(res.results[core]["out"]).astype(np.float32) + const_row)
    return out
